# revision 23
# baseline (speedup 1.0000x reference)
"""Trainium2 Bass kernel for nn_DirectMultiStepModel (2-layer graph-GRU + big Linear + softmax).

Self-contained: takes FULL inputs, shards nodes across 8 NeuronCores internally,
runs a single SPMD NEFF with on-device collectives, returns the FULL (1, 100) output.

v2 strategy:
  - Host: dense normalized adjacency M (10000 -> 10240 padded), fp8 (x64 scaled).
    The final Linear is folded through the layer-2 aggregation on the host:
    W2[o,m,f] = sum_n lin_W[o,n,f] * M[n,m], so the device tail is a single
    column-sharded GEMV against fp8 W2 + AllReduce (no second AllGather/agg).
  - Device per core: all GRU matmuls fp8 with DoubleRow pairing (weights x256,
    rescaled in the activation evict); h1 state kept fp8; per-step AllGather
    (TB=1) of node-major fp8 h1 with a 2-step lag; dense aggregation with
    DoubleRow fp8 in 512-col chunks fused ReLU+bias into fp8 GRU2 input.
"""
import sys
import types
import numpy as np
import ml_dtypes

import concourse.bass as bass
import concourse.bacc as bacc
import concourse.mybir as mybir
import concourse.tile as tile
from concourse.bass_utils import run_bass_kernel_spmd

BF16 = ml_dtypes.bfloat16
E4M3 = ml_dtypes.float8_e4m3
F32 = mybir.dt.float32
BF = mybir.dt.bfloat16
F8 = mybir.dt.float8e4
P = 128

WS = 256.0     # GRU weight scale (fp8 range use)
MS = 64.0      # adjacency scale
W2S = 32768.0  # folded final-linear scale


def _install_ntff_hook():
    """Register the NTFF profile hook the agent image's antenv lacks (no-op if present)."""
    try:
        import antenv.axon_hooks  # noqa: F401
        return
    except ImportError:
        pass
    try:
        import trn_agent_boot.trn_boot as tb
        hooks = types.ModuleType("antenv.axon_hooks")
        _h = [None]
        hooks.set_axon_ntff_profile_hook = lambda h: _h.__setitem__(0, h)
        hooks.get_axon_ntff_profile_hook = lambda: _h[0]
        sys.modules["antenv.axon_hooks"] = hooks
        import antenv
        antenv.axon_hooks = hooks
        hook = tb._ntff_profile_via_ctypes('/opt/axon/libaxon_pjrt.so')
        if hook is not None:
            hooks.set_axon_ntff_profile_hook(hook)
    except Exception:
        pass


class Cfg:
    def __init__(self, T=24, N=10000, DIN=128, H1=256, H2=128, OUT=100, NC=8, LAG=2):
        self.T, self.N, self.DIN, self.H1, self.H2, self.OUT, self.NC = T, N, DIN, H1, H2, OUT, NC
        self.NOWN = -(-N // (NC * P)) * P          # per-core padded node count
        self.NPAD = self.NOWN * NC                 # total padded nodes
        self.NT = self.NOWN // P                   # own node tiles
        self.CT = self.NPAD // P                   # contraction tiles
        self.PS1 = H1 // P                         # h1 feature partition-tiles
        self.PS2 = H2 // P
        self.G1, self.G2 = 3 * H1, 3 * H2
        self.LAG = LAG                             # agg trails GRU1 by LAG steps
        self.HALF = self.NOWN // 2
        self.OPAD = 100                            # per-node output block (4*OPAD must be 16B-aligned)


def fchunks(total, maxf=512):
    out, off = [], 0
    while off < total:
        fl = min(maxf, total - off)
        out.append((off, fl))
        off += fl
    return out


def build(cfg: Cfg):
    """Build + compile the SPMD kernel. Returns the compiled Bacc."""
    c = cfg
    nc = bacc.Bacc("TRN2", target_bir_lowering=False, debug=False, num_devices=c.NC)

    # ---- kernel I/O ----
    xT = nc.dram_tensor("xT", [c.T, c.DIN, c.NOWN], F8, kind="ExternalInput").ap()
    wih1 = nc.dram_tensor("wih1", [c.DIN, c.G1], F8, kind="ExternalInput").ap()
    whh1i = nc.dram_tensor("whh1i", [P, 2 * c.G1], F8, kind="ExternalInput").ap()
    wih2i = nc.dram_tensor("wih2i", [P, 2 * c.G2], F8, kind="ExternalInput").ap()
    whh2 = nc.dram_tensor("whh2", [c.H2, c.G2], BF, kind="ExternalInput").ap()
    b1_rz = nc.dram_tensor("b1_rz", [2 * c.H1, 1], F32, kind="ExternalInput").ap()
    b1_hn = nc.dram_tensor("b1_hn", [c.H1, 1], F32, kind="ExternalInput").ap()
    b1_in = nc.dram_tensor("b1_in", [c.H1, 1], F32, kind="ExternalInput").ap()
    b2_rz = nc.dram_tensor("b2_rz", [2 * c.H2, 1], F32, kind="ExternalInput").ap()
    b2_hn = nc.dram_tensor("b2_hn", [c.H2, 1], F32, kind="ExternalInput").ap()
    b2_in = nc.dram_tensor("b2_in", [c.H2, 1], F32, kind="ExternalInput").ap()
    cb1 = nc.dram_tensor("cb1", [c.H1, 1], F32, kind="ExternalInput").ap()
    mT = nc.dram_tensor("mT", [P, c.CT, c.NOWN], F8, kind="ExternalInput").ap()
    w2p = nc.dram_tensor("w2p", [c.H2, c.HALF // 4, 2, 4 * c.OPAD], F8,
                         kind="ExternalInput").ap()
    c0 = nc.dram_tensor("c0", [1, c.OUT], F32, kind="ExternalInput").ap()
    identf8 = nc.dram_tensor("identf8", [P, P], F8, kind="ExternalInput").ap()
    out = nc.dram_tensor("out", [1, c.OUT], F32, kind="ExternalOutput").ap()

    rg = [list(range(c.NC))]
    AGR = c.PS1 * P                  # payload rows per core per step (node-major tiles)

    Sig = mybir.ActivationFunctionType.Sigmoid
    Tanh = mybir.ActivationFunctionType.Tanh
    Iden = mybir.ActivationFunctionType.Identity
    Relu = mybir.ActivationFunctionType.Relu
    Exp = mybir.ActivationFunctionType.Exp
    Copy = mybir.ActivationFunctionType.Copy
    DR = mybir.MatmulPerfMode.DoubleRow

    with tile.TileContext(nc) as tc:
        with tc.tile_pool(name="dram", bufs=1, space="DRAM") as dram:
            ag_in = dram.tile([c.T, AGR, c.NOWN], F8)
            ag_outs = [dram.tile([AGR * c.NC, c.NOWN], F8, addr_space="Shared",
                                 name=f"ag_out{i}") for i in range(c.T)]
            ar_in = dram.tile([1, c.OUT], F32)
            ar_out = dram.tile([1, c.OUT], F32, addr_space="Shared")

            # ---- constants in SBUF (live for the whole kernel) ----
            with tc.tile_pool(name="const", bufs=1) as cpool:
                wih1_sb = cpool.tile([P, c.G1], F8)
                nc.sync.dma_start(wih1_sb[:], wih1[:])
                whh1i_sb = cpool.tile([P, 2 * c.G1], F8)
                nc.sync.dma_start(whh1i_sb[:], whh1i[:])
                wih2i_sb = cpool.tile([P, 2 * c.G2], F8)
                nc.sync.dma_start(wih2i_sb[:], wih2i[:])
                whh2_sb = cpool.tile([P, c.G2], BF)
                nc.sync.dma_start(whh2_sb[:], whh2[:])
                ident_sb = cpool.tile([P, P], F8)
                nc.sync.dma_start(ident_sb[:], identf8[:])

                def bias_tile(src, n):
                    t = cpool.tile([P, n // P], F32, name=f"b_{src.tensor.name}")
                    for i in range(n // P):
                        nc.sync.dma_start(t[:, i:i + 1], src[i * P:(i + 1) * P, :])
                    return t
                b1rz_sb = bias_tile(b1_rz, 2 * c.H1)
                b1hn_sb = bias_tile(b1_hn, c.H1)
                b1in_sb = bias_tile(b1_in, c.H1)
                b2rz_sb = bias_tile(b2_rz, 2 * c.H2)
                b2hn_sb = bias_tile(b2_hn, c.H2)
                b2in_sb = bias_tile(b2_in, c.H2)
                cb1_sb = bias_tile(cb1, c.H1)
                c0_sb = cpool.tile([1, c.OUT], F32)
                nc.sync.dma_start(c0_sb[:], c0[:])

                GW = c.NOWN // 2      # per-grp node span for GRU pipelining
                FCg = fchunks(GW)     # [(0,512),(512,128)]
                # agg chunks: keep every matmul stream longer than its LDWEIGHTS
                # (a 256-row chunk is shorter than the DR weight load and exposes it)
                ACH = [(0, 512), (512, 384), (896, 384)]

                with tc.tile_pool(name="mtp", bufs=1) as mtp:
                    mt_sb = mtp.tile([P, c.CT * c.NOWN], F8)
                    mt3 = mt_sb[:].rearrange("p (ct f) -> p ct f", f=c.NOWN)

                    with tc.tile_pool(name="p1", bufs=1) as p1, \
                         tc.tile_pool(name="p1x", bufs=3) as p1x, \
                         tc.tile_pool(name="p1w", bufs=2) as p1w, \
                         tc.tile_pool(name="ps1", bufs=2, space="PSUM") as ps1, \
                         tc.tile_pool(name="ps1t", bufs=2, space="PSUM") as ps1t, \
                         tc.tile_pool(name="p1s", bufs=2) as p1s, \
                         tc.tile_pool(name="hstp", bufs=2) as hstp, \
                         tc.tile_pool(name="a1p", bufs=2) as a1p, \
                         tc.tile_pool(name="p3", bufs=1) as p3, \
                         tc.tile_pool(name="p3w", bufs=2) as p3w, \
                         tc.tile_pool(name="aps", bufs=2, space="PSUM") as aps, \
                         tc.tile_pool(name="ps3", bufs=2, space="PSUM") as ps3:
                        h1f8 = p1.tile([P, c.PS1 * c.NOWN], F8)
                        nc.vector.memset(h1f8[:], 0.0)
                        h1p = h1f8[:].rearrange("p (k n) -> p k n", k=2)
                        wh1p = whh1i_sb[:].rearrange("p (k g) -> p k g", k=2)
                        wi2p = wih2i_sb[:].rearrange("p (k g) -> p k g", k=2)
                        h2 = p3.tile([P, c.NOWN], BF)
                        nc.vector.memset(h2[:], 0.0)

                        def gru1_step(xt):
                            for grp in range(2):
                                nofs = grp * GW
                                rz = p1w.tile([P, 2 * c.PS1 * GW], BF, tag="rz")
                                nsb = p1w.tile([P, c.PS1 * GW], BF, tag="nsb")
                                insb = p1w.tile([P, c.PS1 * GW], BF, tag="insb")
                                hnsb = p1w.tile([P, c.PS1 * GW], BF, tag="hnsb")
                                # r,z gates: DoubleRow h-pair + x part, sigmoid evict
                                for g in range(2 * c.PS1):
                                    for (fo, fl) in FCg:
                                        no = nofs + fo
                                        pt = ps1.tile([P, fl], F32, tag="ps_g")
                                        nc.tensor.matmul(pt[:], wh1p[:, :, g * P:(g + 1) * P],
                                                         h1p[:, :, no:no + fl],
                                                         start=True, stop=False,
                                                         perf_mode=DR)
                                        nc.tensor.matmul(pt[:], wih1_sb[:, g * P:(g + 1) * P],
                                                         xt[:, no:no + fl],
                                                         start=False, stop=True)
                                        nc.scalar.activation(
                                            rz[:, g * GW + fo:g * GW + fo + fl], pt[:],
                                            Sig, bias=b1rz_sb[:, g:g + 1], scale=1.0 / WS)
                                # i_n (x part) and h_n (h part, DoubleRow)
                                for g2 in range(c.PS1):
                                    gof = (2 * c.PS1 + g2) * P
                                    for (fo, fl) in FCg:
                                        no = nofs + fo
                                        pi = ps1.tile([P, fl], F32, tag="ps_g")
                                        nc.tensor.matmul(pi[:], wih1_sb[:, gof:gof + P],
                                                         xt[:, no:no + fl],
                                                         start=True, stop=True)
                                        nc.scalar.activation(
                                            insb[:, g2 * GW + fo:g2 * GW + fo + fl], pi[:],
                                            Iden, bias=b1in_sb[:, g2:g2 + 1], scale=1.0 / WS)
                                        ph = ps1.tile([P, fl], F32, tag="ps_g")
                                        nc.tensor.matmul(ph[:], wh1p[:, :, gof:gof + P],
                                                         h1p[:, :, no:no + fl],
                                                         start=True, stop=True,
                                                         perf_mode=DR)
                                        nc.scalar.activation(
                                            hnsb[:, g2 * GW + fo:g2 * GW + fo + fl], ph[:],
                                            Iden, bias=b1hn_sb[:, g2:g2 + 1], scale=1.0 / WS)
                                # gate math: n = tanh(i_n + r*hn); h' = n + z*(h-n)
                                for g2 in range(c.PS1):
                                    sl = slice(g2 * GW, (g2 + 1) * GW)
                                    z_sl = slice((c.PS1 + g2) * GW, (c.PS1 + g2 + 1) * GW)
                                    h_sl = slice(g2 * c.NOWN + nofs, g2 * c.NOWN + nofs + GW)
                                    nc.vector.tensor_mul(hnsb[:, sl], rz[:, sl], hnsb[:, sl])
                                    nc.vector.tensor_add(hnsb[:, sl], hnsb[:, sl], insb[:, sl])
                                    nc.scalar.activation(nsb[:, sl], hnsb[:, sl], Tanh)
                                    nc.gpsimd.tensor_sub(hnsb[:, sl], h1f8[:, h_sl], nsb[:, sl])
                                    nc.vector.tensor_mul(hnsb[:, sl], rz[:, z_sl], hnsb[:, sl])
                                    nc.vector.tensor_add(h1f8[:, h_sl], nsb[:, sl], hnsb[:, sl])

                        def gru2_step(a1t):
                            a1pv = a1t[:].rearrange("p (k n) -> p k n", k=2)
                            for grp in range(2):
                                nofs = grp * GW
                                rz = p3w.tile([P, 2 * GW], BF, tag="rz2")
                                nsb = p3w.tile([P, GW], BF, tag="nsb2")
                                insb = p3w.tile([P, GW], BF, tag="insb2")
                                hnsb = p3w.tile([P, GW], BF, tag="hnsb2")
                                for g in range(2):
                                    for (fo, fl) in FCg:
                                        no = nofs + fo
                                        pt = ps3.tile([P, fl], F32, tag="ps_g2")
                                        nc.tensor.matmul(pt[:], wi2p[:, :, g * P:(g + 1) * P],
                                                         a1pv[:, :, no:no + fl],
                                                         start=True, stop=False,
                                                         perf_mode=DR)
                                        nc.tensor.matmul(pt[:], whh2_sb[:, g * P:(g + 1) * P],
                                                         h2[:, no:no + fl],
                                                         start=False, stop=True)
                                        nc.scalar.activation(
                                            rz[:, g * GW + fo:g * GW + fo + fl], pt[:],
                                            Sig, bias=b2rz_sb[:, g:g + 1], scale=1.0 / WS)
                                gof = 2 * P
                                for (fo, fl) in FCg:
                                    no = nofs + fo
                                    pi = ps3.tile([P, fl], F32, tag="ps_g2")
                                    nc.tensor.matmul(pi[:], wi2p[:, :, gof:gof + P],
                                                     a1pv[:, :, no:no + fl],
                                                     start=True, stop=True, perf_mode=DR)
                                    nc.scalar.activation(insb[:, fo:fo + fl], pi[:],
                                                         Iden, bias=b2in_sb[:, 0:1],
                                                         scale=1.0 / WS)
                                    ph = ps3.tile([P, fl], F32, tag="ps_g2")
                                    nc.tensor.matmul(ph[:], whh2_sb[:, gof:gof + P],
                                                     h2[:, no:no + fl],
                                                     start=True, stop=True)
                                    nc.scalar.activation(hnsb[:, fo:fo + fl], ph[:],
                                                         Iden, bias=b2hn_sb[:, 0:1],
                                                         scale=1.0 / WS)
                                h_sl = slice(nofs, nofs + GW)
                                z_sl = slice(GW, 2 * GW)
                                nc.vector.tensor_mul(hnsb[:], rz[:, 0:GW], hnsb[:])
                                nc.vector.tensor_add(hnsb[:], hnsb[:], insb[:])
                                nc.scalar.activation(nsb[:], hnsb[:], Tanh)
                                nc.gpsimd.tensor_sub(hnsb[:], h2[:, h_sl], nsb[:])
                                nc.vector.tensor_mul(hnsb[:], rz[:, z_sl], hnsb[:])
                                nc.vector.tensor_add(h2[:, h_sl], nsb[:], hnsb[:])

                        def transpose_emitter():
                            # node-major fp8 h1 tiles for the AllGather payload,
                            # emitted one block at a time so the PSUM drains pace
                            # with DVE instead of stalling the PE
                            stgs = [p1s.tile([P, c.NOWN], F8, tag="stg", name=f"stg{i}")
                                    for i in range(c.PS1)]
                            state = [0]

                            def emit_one():
                                i = state[0]
                                if i >= c.PS1 * c.NT:
                                    return
                                ps, nt = divmod(i, c.NT)
                                state[0] = i + 1
                                # fp8 transpose needs output element step 2
                                pt = ps1t.tile([P, 2 * P], F8, tag="ps_t")
                                ptv = pt[:].rearrange("p (f two) -> p f two", two=2)
                                nc.tensor.transpose(
                                    ptv[:, :, 0:1],
                                    h1f8[:, ps * c.NOWN + nt * P:ps * c.NOWN + (nt + 1) * P],
                                    ident_sb[:])
                                nc.vector.tensor_copy(stgs[ps][:, nt * P:(nt + 1) * P],
                                                      ptv[:, :, 0:1])

                            return emit_one, stgs

                        def emit_transposes(t):
                            emit_one, stgs = transpose_emitter()
                            for _ in range(c.PS1 * c.NT):
                                emit_one()
                            return stgs

                        # ===== Unified pipeline: GRU1 + AllGather(lagged agg + GRU2) =====
                        for step in range(c.T + c.LAG):
                            do_gru1 = step < c.T
                            if do_gru1:
                                t = step
                                xt = p1x.tile([P, c.NOWN], F8, tag="xt")
                                nc.sync.dma_start(xt[:], xT[t])
                                gru1_step(xt)
                            stgs = None
                            if step >= c.LAG:
                                u = step - c.LAG
                                if do_gru1:
                                    emit_one, stgs = transpose_emitter()
                                a1t = a1p.tile([P, c.PS1 * c.NOWN], F8, tag="a1")
                                for ps in range(c.PS1):
                                    hst = hstp.tile([P, c.CT * P], F8, tag="hst")
                                    for r in range(c.NC):
                                        ro = (r * c.PS1 + ps) * P
                                        nc.sync.dma_start(hst[:, r * c.NOWN:(r + 1) * c.NOWN],
                                                          ag_outs[u][ro:ro + P, :])
                                    hst3 = hst[:].rearrange("p (ct f) -> p ct f", f=P)
                                    for ci, (co, cl) in enumerate(ACH):
                                        pa = aps.tile([P, cl], F32, tag="pa")
                                        for cp in range(c.CT // 2):
                                            nc.tensor.matmul(
                                                pa[:], hst3[:, 2 * cp:2 * cp + 2, :],
                                                mt3[:, 2 * cp:2 * cp + 2, co:co + cl],
                                                start=(cp == 0), stop=(cp == c.CT // 2 - 1),
                                                perf_mode=DR)
                                            # spread the h1 transposes through the
                                            # second half of the agg stream: gate
                                            # math is long done and each PSUM drain
                                            # hides under ~4 agg matmuls
                                            if do_gru1 and ps == 1 and cp % 2 == 1:
                                                emit_one()
                                        nc.scalar.activation(
                                            a1t[:, ps * c.NOWN + co:ps * c.NOWN + co + cl],
                                            pa[:], Relu, bias=cb1_sb[:, ps:ps + 1],
                                            scale=1.0 / MS)
                                gru2_step(a1t)
                            elif do_gru1:
                                stgs = emit_transposes(step)
                            if do_gru1:
                                t = step
                                for ps in range(c.PS1):
                                    nc.sync.dma_start(ag_in[t, ps * P:(ps + 1) * P, :],
                                                      stgs[ps][:])
                                nc.gpsimd.collective_compute(
                                    "AllGather", mybir.AluOpType.bypass, replica_groups=rg,
                                    ins=[ag_in[t].opt()], outs=[ag_outs[t].opt()])
                                if step < 2:
                                    # adjacency load deferred and issued after the
                                    # staging DMAs so the first AllGathers aren't
                                    # stuck behind 13MB in the DMA queue
                                    hc = c.CT // 2
                                    csl = slice(step * hc * c.NOWN,
                                                (step + 1) * hc * c.NOWN)
                                    nc.sync.dma_start(
                                        mt_sb[:, csl].rearrange("p (ct f) -> p ct f",
                                                                f=c.NOWN),
                                        mT[:, step * hc:(step + 1) * hc])

                    # ---- tail: fold(final linear, layer-2 agg) GEMV + AllReduce ----
                    # 8 nodes per DoubleRow matmul: stationary = 2x4 h2 columns,
                    # moving = the nodes' W2 blocks; block-diagonal accumulation
                    # in a [4, 448] PSUM, diagonal extracted at the end.
                    with tc.tile_pool(name="p4", bufs=1) as p4, \
                         tc.tile_pool(name="p4w", bufs=3) as p4w, \
                         tc.tile_pool(name="ps4", bufs=1, space="PSUM") as ps4:
                        h2f8 = p4.tile([P, c.NOWN], F8)
                        nc.scalar.activation(h2f8[:], h2[:], Copy)
                        h2pv = h2f8[:].rearrange("p (k n) -> p k n", k=2)
                        GB = 4 * c.OPAD
                        NG = c.HALF // 4
                        pacc = ps4.tile([4, GB], F32, tag="pacc")
                        MG = 16
                        for g0 in range(0, NG, MG):
                            lw = p4w.tile([P, MG * 2 * GB], F8, tag="lw")
                            nc.sync.dma_start(
                                lw[:].rearrange("p (g k o) -> p g k o", k=2, o=GB),
                                w2p[:, g0:g0 + MG])
                            lwv = lw[:].rearrange("p (gk o) -> p gk o", o=GB)
                            for gi in range(MG):
                                g = g0 + gi
                                nc.tensor.matmul(pacc[:], h2pv[:, :, 4 * g:4 * g + 4],
                                                 lwv[:, 2 * gi:2 * gi + 2, :],
                                                 start=(g == 0), stop=(g == NG - 1),
                                                 perf_mode=DR)
                        # engines can't start reads at partition>0: evict to SBUF
                        # and fold the 4 diagonal blocks onto partition 0 via DMA
                        sb4 = p4.tile([4, GB], F32)
                        nc.scalar.activation(sb4[:], pacc[:], Copy, scale=1.0 / W2S)
                        diag = p4.tile([1, GB], F32)
                        for j in range(4):
                            nc.sync.dma_start(diag[0:1, j * c.OPAD:(j + 1) * c.OPAD],
                                              sb4[j:j + 1, j * c.OPAD:(j + 1) * c.OPAD])
                        lpart = p4.tile([1, c.OPAD], F32)
                        nc.vector.tensor_add(lpart[:], diag[:, 0:c.OPAD],
                                             diag[:, c.OPAD:2 * c.OPAD])
                        nc.vector.tensor_add(lpart[:], lpart[:],
                                             diag[:, 2 * c.OPAD:3 * c.OPAD])
                        nc.vector.tensor_add(lpart[:], lpart[:],
                                             diag[:, 3 * c.OPAD:4 * c.OPAD])
                        nc.sync.dma_start(ar_in[:], lpart[:, 0:c.OUT])
                        nc.gpsimd.collective_compute(
                            "AllReduce", mybir.AluOpType.add, replica_groups=rg,
                            ins=[ar_in.opt()], outs=[ar_out.opt()])
                        lg = p4.tile([1, c.OUT], F32)
                        nc.sync.dma_start(lg[:], ar_out[:])
                        nc.vector.tensor_add(lg[:], lg[:], c0_sb[:])
                        mx = p4.tile([1, 1], F32)
                        nc.vector.tensor_reduce(mx[:], lg[:], mybir.AxisListType.X,
                                                mybir.AluOpType.max, negate=True)
                        ex = p4.tile([1, c.OUT], F32)
                        nc.scalar.activation(ex[:], lg[:], Exp, bias=mx[:, 0:1])
                        sm = p4.tile([1, 1], F32)
                        nc.vector.tensor_reduce(sm[:], ex[:], mybir.AxisListType.X,
                                                mybir.AluOpType.add)
                        rcp = p4.tile([1, 1], F32)
                        nc.vector.reciprocal(rcp[:], sm[:])
                        res = p4.tile([1, c.OUT], F32)
                        nc.vector.tensor_scalar_mul(res[:], ex[:], rcp[:, 0:1])
                        nc.sync.dma_start(out[:], res[:])

    nc.compile()
    return nc


def host_prep(cfg: Cfg, x, edge_index, W_ih1, W_hh1, b_ih1, b_hh1, bias1,
              W_ih2, W_hh2, b_ih2, b_hh2, bias2, lin_W, lin_b):
    """Shard + preprocess FULL inputs into per-core in_maps."""
    import scipy.sparse as sp
    c = cfg
    x = np.asarray(x, np.float32)
    edge_index = np.asarray(edge_index)
    # dense normalized adjacency, padded: M[dst, src]
    row, col = edge_index[0], edge_index[1]
    loops = np.arange(c.N, dtype=row.dtype)
    row = np.concatenate([row, loops])
    col = np.concatenate([col, loops])
    deg = np.zeros(c.N, np.float32)
    np.add.at(deg, col, 1.0)
    dis = np.where(deg > 0, deg ** -0.5, 0.0).astype(np.float32)
    norm = dis[row] * dis[col]
    M = np.zeros((c.NPAD, c.NPAD), np.float32)
    np.add.at(M, (col, row), norm)

    xp = np.zeros((c.T, c.NPAD, c.DIN), np.float32)
    xp[:, :c.N, :] = x

    def col_f32(v):
        return np.asarray(v, np.float32).reshape(-1, 1)

    def pack_pairs(wT):
        # (2P, G) -> interleaved DoubleRow stationary [P, 2, G] -> [P, 2G]
        g = wT.shape[1]
        return np.ascontiguousarray(
            wT.reshape(2, P, g).transpose(1, 0, 2).reshape(P, 2 * g))

    W_ih1 = np.asarray(W_ih1, np.float32); W_hh1 = np.asarray(W_hh1, np.float32)
    W_ih2 = np.asarray(W_ih2, np.float32); W_hh2 = np.asarray(W_hh2, np.float32)
    b_ih1 = np.asarray(b_ih1, np.float32); b_hh1 = np.asarray(b_hh1, np.float32)
    b_ih2 = np.asarray(b_ih2, np.float32); b_hh2 = np.asarray(b_hh2, np.float32)
    lin_W = np.asarray(lin_W, np.float32)
    lin_b = np.asarray(lin_b, np.float32)
    bias2 = np.asarray(bias2, np.float32)

    # fold final linear through the layer-2 aggregation: W2[of, m] = sum_n W[of, n] M[n, m]
    Msp = sp.csr_matrix(
        (norm.astype(np.float32), (np.asarray(col, np.int64), np.asarray(row, np.int64))),
        shape=(c.N, c.N), dtype=np.float32)
    Wd = lin_W.reshape(c.OUT, c.N, c.H2).transpose(0, 2, 1).reshape(c.OUT * c.H2, c.N)
    W2 = (Msp.T @ Wd.T).T                        # (OUT*H2, N)
    W2p = np.zeros((c.OUT, c.H2, c.NPAD), np.float32)
    W2p[:, :, :c.N] = W2.reshape(c.OUT, c.H2, c.N)
    c0v = (lin_W.reshape(c.OUT, c.N, c.H2).sum(1) @ bias2 + lin_b).reshape(1, c.OUT)

    common = dict(
        wih1=(W_ih1.T * WS).astype(E4M3),
        whh1i=pack_pairs(W_hh1.T * WS).astype(E4M3),
        wih2i=pack_pairs(W_ih2.T * WS).astype(E4M3),
        whh2=(W_hh2.T * WS).astype(BF16),
        b1_rz=col_f32((b_ih1 + b_hh1)[:2 * c.H1]), b1_hn=col_f32(b_hh1[2 * c.H1:]),
        b1_in=col_f32(b_ih1[2 * c.H1:]),
        b2_rz=col_f32((b_ih2 + b_hh2)[:2 * c.H2]), b2_hn=col_f32(b_hh2[2 * c.H2:]),
        b2_in=col_f32(b_ih2[2 * c.H2:]),
        cb1=col_f32(np.asarray(bias1, np.float32)),
        c0=c0v.astype(np.float32),
        identf8=np.eye(P, dtype=E4M3),
    )

    in_maps = []
    for k in range(c.NC):
        sl = slice(k * c.NOWN, (k + 1) * c.NOWN)
        m = dict(common)
        m["xT"] = np.ascontiguousarray(xp[:, sl, :].transpose(0, 2, 1)).astype(E4M3)
        # M^T slice for this core's dest nodes, pre-tiled: (P, CT, NOWN)
        mk = (M[sl, :].T * MS).reshape(c.CT, P, c.NOWN)
        m["mT"] = np.ascontiguousarray(mk.transpose(1, 0, 2)).astype(E4M3)
        # folded linear: [H2, NG, 2, 4*OPAD]; group g, half k, slot j covers
        # node k*HALF + 4g + j with its OPAD-padded output block
        wk = W2p[:, :, sl] * W2S                  # (OUT, H2, NOWN)
        wk = wk.reshape(c.OUT, c.H2, 2, c.HALF // 4, 4)   # (o, f, k, g, j)
        w2k = np.zeros((c.H2, c.HALF // 4, 2, 4, c.OPAD), np.float32)
        w2k[:, :, :, :, :c.OUT] = wk.transpose(1, 3, 2, 4, 0)
        m["w2p"] = w2k.reshape(c.H2, c.HALF // 4, 2, 4 * c.OPAD).astype(E4M3)
        in_maps.append(m)
    return in_maps


_CACHE = {}


def _get_built(key, cfg):
    if key not in _CACHE:
        _CACHE[key] = build(cfg)
    return _CACHE[key]


def run(cfg: Cfg, inputs, trace=False):
    _install_ntff_hook()
    nc = _get_built(("cfg", cfg.T, cfg.N), cfg)
    in_maps = host_prep(cfg, **inputs)
    res = run_bass_kernel_spmd(nc, in_maps, core_ids=list(range(cfg.NC)), trace=trace)
    return res


def kernel(**inputs) -> np.ndarray:
    cfg = Cfg()
    res = run(cfg, inputs)
    return np.asarray(res.results[0]["out"], np.float32)


# revision 27
# speedup vs baseline: 1.0138x; 1.0138x over previous
"""Trainium2 Bass kernel for nn_DirectMultiStepModel (2-layer graph-GRU + big Linear + softmax).

Self-contained: takes FULL inputs, shards nodes across 8 NeuronCores internally,
runs a single SPMD NEFF with on-device collectives, returns the FULL (1, 100) output.

v2 strategy:
  - Host: dense normalized adjacency M (10000 -> 10240 padded), fp8 (x64 scaled).
    The final Linear is folded through the layer-2 aggregation on the host:
    W2[o,m,f] = sum_n lin_W[o,n,f] * M[n,m], so the device tail is a single
    column-sharded GEMV against fp8 W2 + AllReduce (no second AllGather/agg).
  - Device per core: all GRU matmuls fp8 with DoubleRow pairing (weights x256,
    rescaled in the activation evict); h1 state kept fp8; per-step AllGather
    (TB=1) of node-major fp8 h1 with a 2-step lag; dense aggregation with
    DoubleRow fp8 in 512-col chunks fused ReLU+bias into fp8 GRU2 input.
"""
import sys
import types
import numpy as np
import ml_dtypes

import concourse.bass as bass
import concourse.bacc as bacc
import concourse.mybir as mybir
import concourse.tile as tile
from concourse.bass_utils import run_bass_kernel_spmd

BF16 = ml_dtypes.bfloat16
E4M3 = ml_dtypes.float8_e4m3
F32 = mybir.dt.float32
BF = mybir.dt.bfloat16
F8 = mybir.dt.float8e4
P = 128

WS = 256.0     # GRU weight scale (fp8 range use)
MS = 64.0      # adjacency scale
W2S = 32768.0  # folded final-linear scale


def _install_ntff_hook():
    """Register the NTFF profile hook the agent image's antenv lacks (no-op if present)."""
    try:
        import antenv.axon_hooks  # noqa: F401
        return
    except ImportError:
        pass
    try:
        import trn_agent_boot.trn_boot as tb
        hooks = types.ModuleType("antenv.axon_hooks")
        _h = [None]
        hooks.set_axon_ntff_profile_hook = lambda h: _h.__setitem__(0, h)
        hooks.get_axon_ntff_profile_hook = lambda: _h[0]
        sys.modules["antenv.axon_hooks"] = hooks
        import antenv
        antenv.axon_hooks = hooks
        hook = tb._ntff_profile_via_ctypes('/opt/axon/libaxon_pjrt.so')
        if hook is not None:
            hooks.set_axon_ntff_profile_hook(hook)
    except Exception:
        pass


class Cfg:
    def __init__(self, T=24, N=10000, DIN=128, H1=256, H2=128, OUT=100, NC=8, LAG=2):
        self.T, self.N, self.DIN, self.H1, self.H2, self.OUT, self.NC = T, N, DIN, H1, H2, OUT, NC
        self.NOWN = -(-N // (NC * P)) * P          # per-core padded node count
        self.NPAD = self.NOWN * NC                 # total padded nodes
        self.NT = self.NOWN // P                   # own node tiles
        self.CT = self.NPAD // P                   # contraction tiles
        self.PS1 = H1 // P                         # h1 feature partition-tiles
        self.PS2 = H2 // P
        self.G1, self.G2 = 3 * H1, 3 * H2
        self.LAG = LAG                             # agg trails GRU1 by LAG steps
        self.HALF = self.NOWN // 2
        self.OPAD = 100                            # per-node output block (4*OPAD must be 16B-aligned)


def fchunks(total, maxf=512):
    out, off = [], 0
    while off < total:
        fl = min(maxf, total - off)
        out.append((off, fl))
        off += fl
    return out


def build(cfg: Cfg):
    """Build + compile the SPMD kernel. Returns the compiled Bacc."""
    c = cfg
    nc = bacc.Bacc("TRN2", target_bir_lowering=False, debug=False, num_devices=c.NC)

    # ---- kernel I/O ----
    xT = nc.dram_tensor("xT", [c.T, c.DIN, c.NOWN], F8, kind="ExternalInput").ap()
    wih1 = nc.dram_tensor("wih1", [c.DIN, c.G1], F8, kind="ExternalInput").ap()
    whh1i = nc.dram_tensor("whh1i", [P, 2 * c.G1], F8, kind="ExternalInput").ap()
    wih2i = nc.dram_tensor("wih2i", [P, 2 * c.G2], F8, kind="ExternalInput").ap()
    whh2 = nc.dram_tensor("whh2", [c.H2, c.G2], BF, kind="ExternalInput").ap()
    b1_rz = nc.dram_tensor("b1_rz", [2 * c.H1, 1], F32, kind="ExternalInput").ap()
    b1_hn = nc.dram_tensor("b1_hn", [c.H1, 1], F32, kind="ExternalInput").ap()
    b1_in = nc.dram_tensor("b1_in", [c.H1, 1], F32, kind="ExternalInput").ap()
    b2_rz = nc.dram_tensor("b2_rz", [2 * c.H2, 1], F32, kind="ExternalInput").ap()
    b2_hn = nc.dram_tensor("b2_hn", [c.H2, 1], F32, kind="ExternalInput").ap()
    b2_in = nc.dram_tensor("b2_in", [c.H2, 1], F32, kind="ExternalInput").ap()
    cb1 = nc.dram_tensor("cb1", [c.H1, 1], F32, kind="ExternalInput").ap()
    mT = nc.dram_tensor("mT", [P, c.CT, c.NOWN], F8, kind="ExternalInput").ap()
    w2p = nc.dram_tensor("w2p", [c.H2, c.HALF // 4, 2, 4 * c.OPAD], F8,
                         kind="ExternalInput").ap()
    c0 = nc.dram_tensor("c0", [1, c.OUT], F32, kind="ExternalInput").ap()
    identf8 = nc.dram_tensor("identf8", [P, P], F8, kind="ExternalInput").ap()
    out = nc.dram_tensor("out", [1, c.OUT], F32, kind="ExternalOutput").ap()

    rg = [list(range(c.NC))]
    AGR = c.PS1 * P                  # payload rows per core per step (node-major tiles)

    Sig = mybir.ActivationFunctionType.Sigmoid
    Tanh = mybir.ActivationFunctionType.Tanh
    Iden = mybir.ActivationFunctionType.Identity
    Relu = mybir.ActivationFunctionType.Relu
    Exp = mybir.ActivationFunctionType.Exp
    Copy = mybir.ActivationFunctionType.Copy
    DR = mybir.MatmulPerfMode.DoubleRow

    with tile.TileContext(nc) as tc:
        with tc.tile_pool(name="dram", bufs=1, space="DRAM") as dram:
            ag_in = dram.tile([c.T, AGR, c.NOWN], F8)
            ag_outs = [dram.tile([AGR * c.NC, c.NOWN], F8, addr_space="Shared",
                                 name=f"ag_out{i}") for i in range(c.T)]
            ar_in = dram.tile([1, c.OUT], F32)
            ar_out = dram.tile([1, c.OUT], F32, addr_space="Shared")
            warm_in = dram.tile([1, 4], F32)
            warm_out = dram.tile([c.NC, 4], F32, addr_space="Shared")

            # warm up the collective communicator immediately so its init
            # barrier overlaps the first GRU steps instead of gating AG(0)
            nc.gpsimd.collective_compute(
                "AllGather", mybir.AluOpType.bypass, replica_groups=rg,
                ins=[warm_in.opt()], outs=[warm_out.opt()])

            # ---- constants in SBUF (live for the whole kernel) ----
            with tc.tile_pool(name="const", bufs=1) as cpool:
                wih1_sb = cpool.tile([P, c.G1], F8)
                nc.sync.dma_start(wih1_sb[:], wih1[:])
                whh1i_sb = cpool.tile([P, 2 * c.G1], F8)
                nc.sync.dma_start(whh1i_sb[:], whh1i[:])
                wih2i_sb = cpool.tile([P, 2 * c.G2], F8)
                nc.sync.dma_start(wih2i_sb[:], wih2i[:])
                whh2_sb = cpool.tile([P, c.G2], BF)
                nc.sync.dma_start(whh2_sb[:], whh2[:])
                ident_sb = cpool.tile([P, P], F8)
                nc.sync.dma_start(ident_sb[:], identf8[:])

                def bias_tile(src, n):
                    t = cpool.tile([P, n // P], F32, name=f"b_{src.tensor.name}")
                    for i in range(n // P):
                        nc.sync.dma_start(t[:, i:i + 1], src[i * P:(i + 1) * P, :])
                    return t
                b1rz_sb = bias_tile(b1_rz, 2 * c.H1)
                b1hn_sb = bias_tile(b1_hn, c.H1)
                b1in_sb = bias_tile(b1_in, c.H1)
                b2rz_sb = bias_tile(b2_rz, 2 * c.H2)
                b2hn_sb = bias_tile(b2_hn, c.H2)
                b2in_sb = bias_tile(b2_in, c.H2)
                cb1_sb = bias_tile(cb1, c.H1)
                c0_sb = cpool.tile([1, c.OUT], F32)
                nc.sync.dma_start(c0_sb[:], c0[:])

                GW = c.NOWN // 2      # per-grp node span for GRU pipelining
                FCg = fchunks(GW)     # [(0,512),(512,128)]
                # agg chunks: keep every matmul stream longer than its LDWEIGHTS
                # (a 256-row chunk is shorter than the DR weight load and exposes it)
                ACH = [(0, 512), (512, 384), (896, 384)]

                with tc.tile_pool(name="mtp", bufs=1) as mtp:
                    mt_sb = mtp.tile([P, c.CT * c.NOWN], F8)
                    mt3 = mt_sb[:].rearrange("p (ct f) -> p ct f", f=c.NOWN)

                    with tc.tile_pool(name="p1", bufs=1) as p1, \
                         tc.tile_pool(name="p1x", bufs=3) as p1x, \
                         tc.tile_pool(name="p1w", bufs=2) as p1w, \
                         tc.tile_pool(name="ps1", bufs=2, space="PSUM") as ps1, \
                         tc.tile_pool(name="ps1t", bufs=2, space="PSUM") as ps1t, \
                         tc.tile_pool(name="p1s", bufs=2) as p1s, \
                         tc.tile_pool(name="hstp", bufs=2) as hstp, \
                         tc.tile_pool(name="a1p", bufs=2) as a1p, \
                         tc.tile_pool(name="p3", bufs=1) as p3, \
                         tc.tile_pool(name="p3w", bufs=2) as p3w, \
                         tc.tile_pool(name="aps", bufs=2, space="PSUM") as aps, \
                         tc.tile_pool(name="ps3", bufs=2, space="PSUM") as ps3:
                        h1f8 = p1.tile([P, c.PS1 * c.NOWN], F8)
                        nc.vector.memset(h1f8[:], 0.0)
                        h1p = h1f8[:].rearrange("p (k n) -> p k n", k=2)
                        wh1p = whh1i_sb[:].rearrange("p (k g) -> p k g", k=2)
                        wi2p = wih2i_sb[:].rearrange("p (k g) -> p k g", k=2)
                        h2 = p3.tile([P, c.NOWN], BF)
                        nc.vector.memset(h2[:], 0.0)

                        def gru1_step(xt):
                            for grp in range(2):
                                nofs = grp * GW
                                rz = p1w.tile([P, 2 * c.PS1 * GW], BF, tag="rz")
                                nsb = p1w.tile([P, c.PS1 * GW], BF, tag="nsb")
                                insb = p1w.tile([P, c.PS1 * GW], BF, tag="insb")
                                hnsb = p1w.tile([P, c.PS1 * GW], BF, tag="hnsb")
                                # r,z gates: DoubleRow h-pair + x part, sigmoid evict
                                for g in range(2 * c.PS1):
                                    for (fo, fl) in FCg:
                                        no = nofs + fo
                                        pt = ps1.tile([P, fl], F32, tag="ps_g")
                                        nc.tensor.matmul(pt[:], wh1p[:, :, g * P:(g + 1) * P],
                                                         h1p[:, :, no:no + fl],
                                                         start=True, stop=False,
                                                         perf_mode=DR)
                                        nc.tensor.matmul(pt[:], wih1_sb[:, g * P:(g + 1) * P],
                                                         xt[:, no:no + fl],
                                                         start=False, stop=True)
                                        nc.scalar.activation(
                                            rz[:, g * GW + fo:g * GW + fo + fl], pt[:],
                                            Sig, bias=b1rz_sb[:, g:g + 1], scale=1.0 / WS)
                                # i_n (x part) and h_n (h part, DoubleRow)
                                for g2 in range(c.PS1):
                                    gof = (2 * c.PS1 + g2) * P
                                    for (fo, fl) in FCg:
                                        no = nofs + fo
                                        pi = ps1.tile([P, fl], F32, tag="ps_g")
                                        nc.tensor.matmul(pi[:], wih1_sb[:, gof:gof + P],
                                                         xt[:, no:no + fl],
                                                         start=True, stop=True)
                                        nc.scalar.activation(
                                            insb[:, g2 * GW + fo:g2 * GW + fo + fl], pi[:],
                                            Iden, bias=b1in_sb[:, g2:g2 + 1], scale=1.0 / WS)
                                        ph = ps1.tile([P, fl], F32, tag="ps_g")
                                        nc.tensor.matmul(ph[:], wh1p[:, :, gof:gof + P],
                                                         h1p[:, :, no:no + fl],
                                                         start=True, stop=True,
                                                         perf_mode=DR)
                                        nc.scalar.activation(
                                            hnsb[:, g2 * GW + fo:g2 * GW + fo + fl], ph[:],
                                            Iden, bias=b1hn_sb[:, g2:g2 + 1], scale=1.0 / WS)
                                # gate math: n = tanh(i_n + r*hn); h' = n + z*(h-n)
                                for g2 in range(c.PS1):
                                    sl = slice(g2 * GW, (g2 + 1) * GW)
                                    z_sl = slice((c.PS1 + g2) * GW, (c.PS1 + g2 + 1) * GW)
                                    h_sl = slice(g2 * c.NOWN + nofs, g2 * c.NOWN + nofs + GW)
                                    nc.vector.tensor_mul(hnsb[:, sl], rz[:, sl], hnsb[:, sl])
                                    nc.vector.tensor_add(hnsb[:, sl], hnsb[:, sl], insb[:, sl])
                                    nc.scalar.activation(nsb[:, sl], hnsb[:, sl], Tanh)
                                    nc.gpsimd.tensor_sub(hnsb[:, sl], h1f8[:, h_sl], nsb[:, sl])
                                    nc.vector.tensor_mul(hnsb[:, sl], rz[:, z_sl], hnsb[:, sl])
                                    nc.vector.tensor_add(h1f8[:, h_sl], nsb[:, sl], hnsb[:, sl])

                        def gru2_step(a1t):
                            a1pv = a1t[:].rearrange("p (k n) -> p k n", k=2)
                            for grp in range(2):
                                nofs = grp * GW
                                rz = p3w.tile([P, 2 * GW], BF, tag="rz2")
                                nsb = p3w.tile([P, GW], BF, tag="nsb2")
                                insb = p3w.tile([P, GW], BF, tag="insb2")
                                hnsb = p3w.tile([P, GW], BF, tag="hnsb2")
                                for g in range(2):
                                    for (fo, fl) in FCg:
                                        no = nofs + fo
                                        pt = ps3.tile([P, fl], F32, tag="ps_g2")
                                        nc.tensor.matmul(pt[:], wi2p[:, :, g * P:(g + 1) * P],
                                                         a1pv[:, :, no:no + fl],
                                                         start=True, stop=False,
                                                         perf_mode=DR)
                                        nc.tensor.matmul(pt[:], whh2_sb[:, g * P:(g + 1) * P],
                                                         h2[:, no:no + fl],
                                                         start=False, stop=True)
                                        nc.scalar.activation(
                                            rz[:, g * GW + fo:g * GW + fo + fl], pt[:],
                                            Sig, bias=b2rz_sb[:, g:g + 1], scale=1.0 / WS)
                                gof = 2 * P
                                for (fo, fl) in FCg:
                                    no = nofs + fo
                                    pi = ps3.tile([P, fl], F32, tag="ps_g2")
                                    nc.tensor.matmul(pi[:], wi2p[:, :, gof:gof + P],
                                                     a1pv[:, :, no:no + fl],
                                                     start=True, stop=True, perf_mode=DR)
                                    nc.scalar.activation(insb[:, fo:fo + fl], pi[:],
                                                         Iden, bias=b2in_sb[:, 0:1],
                                                         scale=1.0 / WS)
                                    ph = ps3.tile([P, fl], F32, tag="ps_g2")
                                    nc.tensor.matmul(ph[:], whh2_sb[:, gof:gof + P],
                                                     h2[:, no:no + fl],
                                                     start=True, stop=True)
                                    nc.scalar.activation(hnsb[:, fo:fo + fl], ph[:],
                                                         Iden, bias=b2hn_sb[:, 0:1],
                                                         scale=1.0 / WS)
                                h_sl = slice(nofs, nofs + GW)
                                z_sl = slice(GW, 2 * GW)
                                nc.vector.tensor_mul(hnsb[:], rz[:, 0:GW], hnsb[:])
                                nc.vector.tensor_add(hnsb[:], hnsb[:], insb[:])
                                nc.scalar.activation(nsb[:], hnsb[:], Tanh)
                                nc.gpsimd.tensor_sub(hnsb[:], h2[:, h_sl], nsb[:])
                                nc.vector.tensor_mul(hnsb[:], rz[:, z_sl], hnsb[:])
                                nc.vector.tensor_add(h2[:, h_sl], nsb[:], hnsb[:])

                        def transpose_emitter():
                            # node-major fp8 h1 tiles for the AllGather payload,
                            # emitted one block at a time so the PSUM drains pace
                            # with DVE instead of stalling the PE
                            stgs = [p1s.tile([P, c.NOWN], F8, tag="stg", name=f"stg{i}")
                                    for i in range(c.PS1)]
                            state = [0]

                            def emit_one():
                                i = state[0]
                                if i >= c.PS1 * c.NT:
                                    return
                                ps, nt = divmod(i, c.NT)
                                state[0] = i + 1
                                # fp8 transpose needs output element step 2
                                pt = ps1t.tile([P, 2 * P], F8, tag="ps_t")
                                ptv = pt[:].rearrange("p (f two) -> p f two", two=2)
                                nc.tensor.transpose(
                                    ptv[:, :, 0:1],
                                    h1f8[:, ps * c.NOWN + nt * P:ps * c.NOWN + (nt + 1) * P],
                                    ident_sb[:])
                                nc.vector.tensor_copy(stgs[ps][:, nt * P:(nt + 1) * P],
                                                      ptv[:, :, 0:1])

                            return emit_one, stgs

                        def emit_transposes(t):
                            emit_one, stgs = transpose_emitter()
                            for _ in range(c.PS1 * c.NT):
                                emit_one()
                            return stgs

                        # ===== Unified pipeline: GRU1 + AllGather(lagged agg + GRU2) =====
                        for step in range(c.T + c.LAG):
                            do_gru1 = step < c.T
                            if do_gru1:
                                t = step
                                xt = p1x.tile([P, c.NOWN], F8, tag="xt")
                                nc.sync.dma_start(xt[:], xT[t])
                                gru1_step(xt)
                            stgs = None
                            if step >= c.LAG:
                                u = step - c.LAG
                                if do_gru1:
                                    emit_one, stgs = transpose_emitter()
                                a1t = a1p.tile([P, c.PS1 * c.NOWN], F8, tag="a1")
                                for ps in range(c.PS1):
                                    hst = hstp.tile([P, c.CT * P], F8, tag="hst")
                                    for r in range(c.NC):
                                        ro = (r * c.PS1 + ps) * P
                                        nc.sync.dma_start(hst[:, r * c.NOWN:(r + 1) * c.NOWN],
                                                          ag_outs[u][ro:ro + P, :])
                                    hst3 = hst[:].rearrange("p (ct f) -> p ct f", f=P)
                                    for ci, (co, cl) in enumerate(ACH):
                                        pa = aps.tile([P, cl], F32, tag="pa")
                                        for cp in range(c.CT // 2):
                                            nc.tensor.matmul(
                                                pa[:], hst3[:, 2 * cp:2 * cp + 2, :],
                                                mt3[:, 2 * cp:2 * cp + 2, co:co + cl],
                                                start=(cp == 0), stop=(cp == c.CT // 2 - 1),
                                                perf_mode=DR)
                                            # spread the h1 transposes through the
                                            # second half of the agg stream: gate
                                            # math is long done and each PSUM drain
                                            # hides under ~4 agg matmuls
                                            if do_gru1 and ps == 1 and cp % 2 == 1:
                                                emit_one()
                                        nc.scalar.activation(
                                            a1t[:, ps * c.NOWN + co:ps * c.NOWN + co + cl],
                                            pa[:], Relu, bias=cb1_sb[:, ps:ps + 1],
                                            scale=1.0 / MS)
                                gru2_step(a1t)
                            elif do_gru1:
                                stgs = emit_transposes(step)
                            if do_gru1:
                                t = step
                                for ps in range(c.PS1):
                                    nc.sync.dma_start(ag_in[t, ps * P:(ps + 1) * P, :],
                                                      stgs[ps][:])
                                nc.gpsimd.collective_compute(
                                    "AllGather", mybir.AluOpType.bypass, replica_groups=rg,
                                    ins=[ag_in[t].opt()], outs=[ag_outs[t].opt()])
                                if step < 2:
                                    # adjacency load deferred and issued after the
                                    # staging DMAs so the first AllGathers aren't
                                    # stuck behind 13MB in the DMA queue
                                    hc = c.CT // 2
                                    csl = slice(step * hc * c.NOWN,
                                                (step + 1) * hc * c.NOWN)
                                    nc.sync.dma_start(
                                        mt_sb[:, csl].rearrange("p (ct f) -> p ct f",
                                                                f=c.NOWN),
                                        mT[:, step * hc:(step + 1) * hc])

                    # ---- tail: fold(final linear, layer-2 agg) GEMV + AllReduce ----
                    # 8 nodes per DoubleRow matmul: stationary = 2x4 h2 columns,
                    # moving = the nodes' W2 blocks; block-diagonal accumulation
                    # in a [4, 448] PSUM, diagonal extracted at the end.
                    with tc.tile_pool(name="p4", bufs=1) as p4, \
                         tc.tile_pool(name="p4w", bufs=3) as p4w, \
                         tc.tile_pool(name="ps4", bufs=1, space="PSUM") as ps4:
                        h2f8 = p4.tile([P, c.NOWN], F8)
                        nc.scalar.activation(h2f8[:], h2[:], Copy)
                        h2pv = h2f8[:].rearrange("p (k n) -> p k n", k=2)
                        GB = 4 * c.OPAD
                        NG = c.HALF // 4
                        pacc = ps4.tile([4, GB], F32, tag="pacc")
                        MG = 16
                        for g0 in range(0, NG, MG):
                            lw = p4w.tile([P, MG * 2 * GB], F8, tag="lw")
                            nc.sync.dma_start(
                                lw[:].rearrange("p (g k o) -> p g k o", k=2, o=GB),
                                w2p[:, g0:g0 + MG])
                            lwv = lw[:].rearrange("p (gk o) -> p gk o", o=GB)
                            for gi in range(MG):
                                g = g0 + gi
                                nc.tensor.matmul(pacc[:], h2pv[:, :, 4 * g:4 * g + 4],
                                                 lwv[:, 2 * gi:2 * gi + 2, :],
                                                 start=(g == 0), stop=(g == NG - 1),
                                                 perf_mode=DR)
                        # engines can't start reads at partition>0: evict to SBUF
                        # and fold the 4 diagonal blocks onto partition 0 via DMA
                        sb4 = p4.tile([4, GB], F32)
                        nc.scalar.activation(sb4[:], pacc[:], Copy, scale=1.0 / W2S)
                        diag = p4.tile([1, GB], F32)
                        for j in range(4):
                            nc.sync.dma_start(diag[0:1, j * c.OPAD:(j + 1) * c.OPAD],
                                              sb4[j:j + 1, j * c.OPAD:(j + 1) * c.OPAD])
                        lpart = p4.tile([1, c.OPAD], F32)
                        nc.vector.tensor_add(lpart[:], diag[:, 0:c.OPAD],
                                             diag[:, c.OPAD:2 * c.OPAD])
                        nc.vector.tensor_add(lpart[:], lpart[:],
                                             diag[:, 2 * c.OPAD:3 * c.OPAD])
                        nc.vector.tensor_add(lpart[:], lpart[:],
                                             diag[:, 3 * c.OPAD:4 * c.OPAD])
                        nc.sync.dma_start(ar_in[:], lpart[:, 0:c.OUT])
                        nc.gpsimd.collective_compute(
                            "AllReduce", mybir.AluOpType.add, replica_groups=rg,
                            ins=[ar_in.opt()], outs=[ar_out.opt()])
                        lg = p4.tile([1, c.OUT], F32)
                        nc.sync.dma_start(lg[:], ar_out[:])
                        nc.vector.tensor_add(lg[:], lg[:], c0_sb[:])
                        mx = p4.tile([1, 1], F32)
                        nc.vector.tensor_reduce(mx[:], lg[:], mybir.AxisListType.X,
                                                mybir.AluOpType.max, negate=True)
                        ex = p4.tile([1, c.OUT], F32)
                        nc.scalar.activation(ex[:], lg[:], Exp, bias=mx[:, 0:1])
                        sm = p4.tile([1, 1], F32)
                        nc.vector.tensor_reduce(sm[:], ex[:], mybir.AxisListType.X,
                                                mybir.AluOpType.add)
                        rcp = p4.tile([1, 1], F32)
                        nc.vector.reciprocal(rcp[:], sm[:])
                        res = p4.tile([1, c.OUT], F32)
                        nc.vector.tensor_scalar_mul(res[:], ex[:], rcp[:, 0:1])
                        nc.sync.dma_start(out[:], res[:])

    nc.compile()
    return nc


def host_prep(cfg: Cfg, x, edge_index, W_ih1, W_hh1, b_ih1, b_hh1, bias1,
              W_ih2, W_hh2, b_ih2, b_hh2, bias2, lin_W, lin_b):
    """Shard + preprocess FULL inputs into per-core in_maps."""
    import scipy.sparse as sp
    c = cfg
    x = np.asarray(x, np.float32)
    edge_index = np.asarray(edge_index)
    # dense normalized adjacency, padded: M[dst, src]
    row, col = edge_index[0], edge_index[1]
    loops = np.arange(c.N, dtype=row.dtype)
    row = np.concatenate([row, loops])
    col = np.concatenate([col, loops])
    deg = np.zeros(c.N, np.float32)
    np.add.at(deg, col, 1.0)
    dis = np.where(deg > 0, deg ** -0.5, 0.0).astype(np.float32)
    norm = dis[row] * dis[col]
    M = np.zeros((c.NPAD, c.NPAD), np.float32)
    np.add.at(M, (col, row), norm)

    xp = np.zeros((c.T, c.NPAD, c.DIN), np.float32)
    xp[:, :c.N, :] = x

    def col_f32(v):
        return np.asarray(v, np.float32).reshape(-1, 1)

    def pack_pairs(wT):
        # (2P, G) -> interleaved DoubleRow stationary [P, 2, G] -> [P, 2G]
        g = wT.shape[1]
        return np.ascontiguousarray(
            wT.reshape(2, P, g).transpose(1, 0, 2).reshape(P, 2 * g))

    W_ih1 = np.asarray(W_ih1, np.float32); W_hh1 = np.asarray(W_hh1, np.float32)
    W_ih2 = np.asarray(W_ih2, np.float32); W_hh2 = np.asarray(W_hh2, np.float32)
    b_ih1 = np.asarray(b_ih1, np.float32); b_hh1 = np.asarray(b_hh1, np.float32)
    b_ih2 = np.asarray(b_ih2, np.float32); b_hh2 = np.asarray(b_hh2, np.float32)
    lin_W = np.asarray(lin_W, np.float32)
    lin_b = np.asarray(lin_b, np.float32)
    bias2 = np.asarray(bias2, np.float32)

    # fold final linear through the layer-2 aggregation: W2[of, m] = sum_n W[of, n] M[n, m]
    Msp = sp.csr_matrix(
        (norm.astype(np.float32), (np.asarray(col, np.int64), np.asarray(row, np.int64))),
        shape=(c.N, c.N), dtype=np.float32)
    Wd = lin_W.reshape(c.OUT, c.N, c.H2).transpose(0, 2, 1).reshape(c.OUT * c.H2, c.N)
    W2 = (Msp.T @ Wd.T).T                        # (OUT*H2, N)
    W2p = np.zeros((c.OUT, c.H2, c.NPAD), np.float32)
    W2p[:, :, :c.N] = W2.reshape(c.OUT, c.H2, c.N)
    c0v = (lin_W.reshape(c.OUT, c.N, c.H2).sum(1) @ bias2 + lin_b).reshape(1, c.OUT)

    common = dict(
        wih1=(W_ih1.T * WS).astype(E4M3),
        whh1i=pack_pairs(W_hh1.T * WS).astype(E4M3),
        wih2i=pack_pairs(W_ih2.T * WS).astype(E4M3),
        whh2=(W_hh2.T * WS).astype(BF16),
        b1_rz=col_f32((b_ih1 + b_hh1)[:2 * c.H1]), b1_hn=col_f32(b_hh1[2 * c.H1:]),
        b1_in=col_f32(b_ih1[2 * c.H1:]),
        b2_rz=col_f32((b_ih2 + b_hh2)[:2 * c.H2]), b2_hn=col_f32(b_hh2[2 * c.H2:]),
        b2_in=col_f32(b_ih2[2 * c.H2:]),
        cb1=col_f32(np.asarray(bias1, np.float32)),
        c0=c0v.astype(np.float32),
        identf8=np.eye(P, dtype=E4M3),
    )

    in_maps = []
    for k in range(c.NC):
        sl = slice(k * c.NOWN, (k + 1) * c.NOWN)
        m = dict(common)
        m["xT"] = np.ascontiguousarray(xp[:, sl, :].transpose(0, 2, 1)).astype(E4M3)
        # M^T slice for this core's dest nodes, pre-tiled: (P, CT, NOWN)
        mk = (M[sl, :].T * MS).reshape(c.CT, P, c.NOWN)
        m["mT"] = np.ascontiguousarray(mk.transpose(1, 0, 2)).astype(E4M3)
        # folded linear: [H2, NG, 2, 4*OPAD]; group g, half k, slot j covers
        # node k*HALF + 4g + j with its OPAD-padded output block
        wk = W2p[:, :, sl] * W2S                  # (OUT, H2, NOWN)
        wk = wk.reshape(c.OUT, c.H2, 2, c.HALF // 4, 4)   # (o, f, k, g, j)
        w2k = np.zeros((c.H2, c.HALF // 4, 2, 4, c.OPAD), np.float32)
        w2k[:, :, :, :, :c.OUT] = wk.transpose(1, 3, 2, 4, 0)
        m["w2p"] = w2k.reshape(c.H2, c.HALF // 4, 2, 4 * c.OPAD).astype(E4M3)
        in_maps.append(m)
    return in_maps


_CACHE = {}


def _get_built(key, cfg):
    if key not in _CACHE:
        _CACHE[key] = build(cfg)
    return _CACHE[key]


def run(cfg: Cfg, inputs, trace=False):
    _install_ntff_hook()
    nc = _get_built(("cfg", cfg.T, cfg.N), cfg)
    in_maps = host_prep(cfg, **inputs)
    res = run_bass_kernel_spmd(nc, in_maps, core_ids=list(range(cfg.NC)), trace=trace)
    return res


def kernel(**inputs) -> np.ndarray:
    cfg = Cfg()
    res = run(cfg, inputs)
    return np.asarray(res.results[0]["out"], np.float32)
